# revision 16
# baseline (speedup 1.0000x reference)
"""ChannelAttention Trainium2 kernel.

Reference computation (per batch b, group o):
    p_mean[s, c] = mean over (h, w) of x[b, o, s, c, :, :]
    p_max[s, c]  = max  over (h, w) of x[b, o, s, c, :, :]
    out = sigmoid(relu(p_mean @ w1[o].T) @ w2[o].T + relu(p_max @ w1[o].T) @ w2[o].T)
    result[b, o, s, c, 0, 0] = out[s, c]

Strategy: data-parallel over batch B=8 -> one batch per NeuronCore (64 MiB
of x per core; the kernel is HBM-bandwidth bound on streaming x, ~392 GB/s
sustained per core = ~171 us).

Per core, x[b] is viewed as [O*S*C, H*W] = [16384, 1024] and streamed in
4 MiB tiles of [128 partitions, 8*1024] on the sync HWDGE ring. Each
128-row block covers 2 s-values x 64 channels, so pooled results land as
[partition = (s%2)*64 + c, column o*32 + path*16 + s//2] (path 0 = mean,
1 = max, adjacent so one matmul consumes both).

Reductions use two custom DVE ops (registered below via the per-NEFF
ucode-table path; the native TENSOR_TENSOR_REDUCE opcode faults this
runtime's DVE sequencer): feeding the two 512-element halves of a block
as in0/in1 uses both SBUF read ports, so a 1024-element row reduces in
~512 DVE cycles (~0.69 us/block vs 1.21 for plain tensor_reduce). The
mean op applies scale=1/1024 inline, which lets the mean and max paths
share one unscaled FC1 weight block. All maxes plus 3/8 of the means
run on DVE (~123 us); the remaining means run on the scalar engine as
activation-Copy-accumulate (~121 us), so both reduce engines sit ~50 us
under the DMA stream and never backlog into the kernel tail.

The tiny grouped MLP uses block-diagonal duplicated weights
([[W.T, 0], [0, W.T]], built host-side): per group ONE 128x128x32 matmul
over the adjacent [mean|max] columns, relu, a DVE add of the two halves
(relu(mean@w1T) + relu(max@w1T)), one FC2 matmul with the summed hidden
as stationary, sigmoid, and a single batched store of all groups at the
end (the sync ring stays dedicated to the x stream).
"""

from operator import add as _op_add

import numpy as np

import concourse.bacc as bacc
import concourse.bass as bass
import concourse.dve_ops as dve_ops
import concourse.mybir as mybir
import concourse.tile as tile
from concourse.bass_utils import run_bass_kernel_spmd
from concourse.dve_spec import (
    C0,
    MaxNeg,
    Spec,
    Src0,
    Src1,
    Zero,
    _has_src1,
    lower,
    maxx,
)
from concourse.dve_uop import DveOpSpec

B, O, S, C, H, W = 8, 8, 32, 64, 32, 32
HID = C
HWSZ = H * W            # 1024 elements pooled per (b, o, s, c)
ROWS = O * S * C        # 16384 rows per core
RB = 128                # rows per partition block
T = ROWS // RB          # 128 row-blocks per core
JB = 8                  # row-blocks per stream tile (4 MiB DMAs)
HH = HWSZ // 2          # half-block length for the 2-port DVE reduce
SP = S // 2             # 16 pooled columns per (group, path)
N_CORES = 8

_CACHE = {}


# --- custom DVE pair-reduce ops -------------------------------------------


def _ref_pair_max(in0, in1, c0, c1, c2):
    b = np.maximum(in0.astype(np.float32), in1).astype(np.float32)
    return b, b.reshape(b.shape[0], -1).max(axis=-1, keepdims=True)


def _ref_pair_mean(in0, in1, c0, c1, c2):
    b = ((in0.astype(np.float32) + in1) * c0).astype(np.float32)
    return b, b.reshape(b.shape[0], -1).sum(axis=-1, keepdims=True)


def _make_op(name, spec):
    # Pin uops_sha by lowering locally for both ucode versions (the sha
    # covers only the serialized uop table bytes, not the opcode row).
    shas = {}
    for ver in ("v3", "v4"):
        tmp = DveOpSpec(
            name=name, opcode=0, uops=lower(spec, ver=ver), rd1_en=_has_src1(spec)
        )
        shas[ver] = tmp.sha(ver)
    return dve_ops.DveOp(name, spec, subdim=False, uops_sha=shas)


def _register_pair_ops():
    existing = {op.name: op for op in dve_ops.OPS}
    if "PAIR_MAX_REDUCE_ANT" in existing:
        return existing["PAIR_MAX_REDUCE_ANT"], existing["PAIR_MEAN_REDUCE_ANT"]
    pmax = _make_op(
        "PAIR_MAX_REDUCE_ANT",
        Spec(
            body=maxx(Src0, Src1),
            accum=maxx,
            accum_init=MaxNeg,
            reference=_ref_pair_max,
        ),
    )
    pmean = _make_op(
        "PAIR_MEAN_REDUCE_ANT",
        Spec(
            body=(Src0 + Src1) * C0,
            accum=_op_add,
            accum_init=Zero,
            reference=_ref_pair_mean,
        ),
    )
    for op in (pmax, pmean):
        dve_ops.OPS.append(op)
        dve_ops._SUB_OPCODE_FOR_NAME[op.name] = (
            max(dve_ops._SUB_OPCODE_FOR_NAME.values()) + 1
        )
        dve_ops.CUSTOM_DVE_SPECS[op.name] = op.spec
    assert max(dve_ops._SUB_OPCODE_FOR_NAME.values()) < 0x20
    return pmax, pmean


PMAX, PMEAN = _register_pair_ops()


def _build_nc():
    nc = bacc.Bacc(
        "TRN2", target_bir_lowering=False, debug=False, num_devices=N_CORES
    )
    x = nc.dram_tensor("x", [ROWS, HWSZ], mybir.dt.float32, kind="ExternalInput")
    wdup = nc.dram_tensor(
        "wdup", [128, 2 * O * 128], mybir.dt.float32, kind="ExternalInput"
    )
    out = nc.dram_tensor("out", [O * S, C], mybir.dt.float32, kind="ExternalOutput")

    fp32 = mybir.dt.float32
    AF = mybir.ActivationFunctionType
    ALU = mybir.AluOpType

    with tile.TileContext(nc) as tc:
        with (
            tc.tile_pool(name="xp", bufs=4) as xp,
            tc.tile_pool(name="tp", bufs=5) as tp,
            tc.tile_pool(name="small", bufs=1) as sp,
            tc.tile_pool(name="psum1", bufs=1, space=bass.MemorySpace.PSUM) as pp1,
            tc.tile_pool(name="psum2", bufs=1, space=bass.MemorySpace.PSUM) as pp2,
        ):
            # Weight load and the output store go through the scalar HWDGE
            # ring: the sync HWDGE ring executes FIFO per engine, so anything
            # else there would head-of-line block the x stream. GpSimd issues
            # no instructions at all.
            wd = sp.tile([128, 2 * O * 128], fp32)
            nc.scalar.dma_start(wd[:], wdup.ap())

            # pooled[:, o*32 + path*16 + j]: path 0 = mean, path 1 = max.
            pooled = sp.tile([128, 2 * T], fp32)
            junk = sp.tile([128, HWSZ], fp32)
            vjunk = sp.tile([128, HH], fp32)
            h_sb = sp.tile([128, 32 * O], fp32)
            hs_sb = sp.tile([128, SP * O], fp32)
            att = sp.tile([SP, O * 128], fp32)

            xv = x.ap().rearrange("(t p) f -> t p f", p=RB)
            # Output views for the stores: per partition j, one 512 B
            # descriptor per group (the r,c rows are contiguous in DRAM).
            ov = out.ap().rearrange("(o j r) c -> o j r c", o=O, j=SP, r=2)
            av = att.rearrange("p (o r c) -> o p r c", o=O, r=2)

            def colm(t):  # pooled column of block t's mean
                return (t // SP) * 32 + (t % SP)

            def colx(t):  # pooled column of block t's max
                return (t // SP) * 32 + SP + (t % SP)

            def dve_max(xt, j, t):
                nc.vector._custom_dve(
                    PMAX,
                    out=vjunk[:],
                    in0=xt[:, j, 0:HH],
                    in1=xt[:, j, HH:HWSZ],
                    accum_out=pooled[:, colx(t) : colx(t) + 1],
                )

            def dve_mean(xt, j, t):
                nc.vector._custom_dve(
                    PMEAN,
                    out=vjunk[:],
                    in0=xt[:, j, 0:HH],
                    in1=xt[:, j, HH:HWSZ],
                    s0=1.0 / HWSZ,
                    accum_out=pooled[:, colm(t) : colm(t) + 1],
                )

            def act_mean(xt, j, t):
                nc.scalar.activation(
                    junk[:],
                    xt[:, j, :],
                    AF.Copy,
                    scale=1.0 / HWSZ,
                    accum_out=pooled[:, colm(t) : colm(t) + 1],
                )

            def mlp(o):
                w1b = wd[:, o * 128 : (o + 1) * 128]
                w2b = wd[:, O * 128 + o * 128 : O * 128 + (o + 1) * 128]
                ps1 = pp1.tile([128, 32], fp32, tag="ps1")
                nc.tensor.matmul(ps1[:], w1b, pooled[:, o * 32 : (o + 1) * 32])
                h = h_sb[:, o * 32 : (o + 1) * 32]
                nc.scalar.activation(h, ps1[:], AF.Relu)
                hs = hs_sb[:, o * SP : (o + 1) * SP]
                nc.vector.tensor_tensor(hs, h[:, 0:SP], h[:, SP:32], op=ALU.add)
                ps2 = pp2.tile([SP, 128], fp32, tag="ps2")
                nc.tensor.matmul(ps2[:], hs, w2b)
                nc.scalar.activation(att[:, o * 128 : (o + 1) * 128], ps2[:], AF.Sigmoid)

            # 4 MiB DMAs for the bulk of the stream, tapering to 512 KiB at
            # the end so the final reduce->MLP->store chain starts on less
            # data (shorter kernel tail).
            chunks = [8] * 15 + [2, 2, 2, 1, 1]
            assert sum(chunks) == T
            t0 = 0
            for i, jb in enumerate(chunks):
                # Taper chunks use their own (fresh) pool so their DMAs are
                # not gated on the big chunks' reduce bursts freeing buffers.
                if jb == 8:
                    xt = xp.tile([RB, JB, HWSZ], fp32, tag="xt")
                else:
                    xt = tp.tile([RB, 2, HWSZ], fp32, tag="xtt")
                nc.sync.dma_start(
                    xt[:, :jb, :],
                    xv[t0 : t0 + jb].transpose([1, 0, 2]),
                )
                for j in range(jb):
                    dve_max(xt, j, t0 + j)
                # Means: DVE takes 3 of 8 per full chunk (2 of 8 late in the
                # stream, so its queue is drained when the taper arrives) and
                # one per 2-block taper chunk; the scalar engine takes the
                # rest, including the final single blocks so the last block's
                # mean and max run on different engines in parallel.
                for j in range(jb):
                    t = t0 + j
                    if (
                        (jb == 8 and i < 10 and j % 3 == 0)
                        or (jb == 8 and i >= 10 and j % 2 == 1)
                        or (jb == 2 and j == 0)
                        or t == T - 2
                    ):
                        # Late in the stream DVE takes 4 of 8 means (its
                        # per-op cost is 0.81 us vs 1.43 on ACT), so neither
                        # engine has a backlog when the last block lands;
                        # t == T-2 on DVE so the final block's mean (ACT)
                        # and max (DVE) finish in parallel at stream end.
                        dve_mean(xt, j, t)
                    else:
                        act_mean(xt, j, t)
                # Group o's pooled columns are complete once row-blocks
                # through t = (o+1)*SP - 1 are reduced; emit its MLP as soon
                # as that happens so it overlaps the remaining stream.
                done = t0 + jb
                for o in range(O):
                    if t0 < (o + 1) * SP <= done:
                        mlp(o)
                        # Stores ride the scalar HWDGE ring (lower completion
                        # latency than SWDGE; the sync ring stays dedicated
                        # to the x stream). Groups 0-6 go out in one batched
                        # store that overlaps the tail of the stream; only
                        # group 7's 8 KiB store sits on the critical path
                        # after the final sigmoid.
                        if o == 6:
                            nc.scalar.dma_start(
                                ov[0:7].transpose([1, 0, 2, 3]),
                                av[0:7].transpose([1, 0, 2, 3]),
                            )
                        elif o == 7:
                            nc.scalar.dma_start(ov[7], av[7])
                t0 = done

    nc.compile()
    return nc


def _build_wdup(w1, w2):
    # Two sections of 8 block-diagonal duplicated 128x128 matrices: w1.T
    # (shared by the mean and max paths -- the mean is pre-scaled during
    # reduction) and w2.T.
    wdup = np.zeros((128, 2 * O * 128), dtype=np.float32)
    for o in range(O):
        w1t = np.ascontiguousarray(w1[o].T)  # [C, HID]
        w2t = np.ascontiguousarray(w2[o].T)  # [HID, C]
        for sec, blk in ((0, w1t), (1, w2t)):
            base = sec * O * 128 + o * 128
            wdup[0:64, base : base + 64] = blk
            wdup[64:128, base + 64 : base + 128] = blk
    return wdup


def kernel(x, w1, w2):
    if "nc" not in _CACHE:
        _CACHE["nc"] = _build_nc()
    nc = _CACHE["nc"]

    x = np.ascontiguousarray(x, dtype=np.float32).reshape(B, ROWS, HWSZ)
    wdup = _build_wdup(
        np.asarray(w1, dtype=np.float32), np.asarray(w2, dtype=np.float32)
    )
    in_maps = [{"x": x[b], "wdup": wdup} for b in range(B)]
    res = run_bass_kernel_spmd(nc, in_maps, core_ids=list(range(N_CORES)))
    out = np.stack([res.results[b]["out"] for b in range(B)])
    return out.reshape(B, O, S, C, 1, 1).astype(np.float32)


# revision 20
# speedup vs baseline: 1.0126x; 1.0126x over previous
"""ChannelAttention Trainium2 kernel.

Reference computation (per batch b, group o):
    p_mean[s, c] = mean over (h, w) of x[b, o, s, c, :, :]
    p_max[s, c]  = max  over (h, w) of x[b, o, s, c, :, :]
    out = sigmoid(relu(p_mean @ w1[o].T) @ w2[o].T + relu(p_max @ w1[o].T) @ w2[o].T)
    result[b, o, s, c, 0, 0] = out[s, c]

Strategy: data-parallel over batch B=8 -> one batch per NeuronCore (64 MiB
of x per core; the kernel is HBM-bandwidth bound on streaming x, ~392 GB/s
sustained per core = ~171 us).

Per core, x[b] is viewed as [O*S*C, H*W] = [16384, 1024] and streamed in
4 MiB tiles of [128 partitions, 8*1024] on the sync HWDGE ring. Each
128-row block covers 2 s-values x 64 channels, so pooled results land as
[partition = (s%2)*64 + c, column o*32 + path*16 + s//2] (path 0 = mean,
1 = max, adjacent so one matmul consumes both).

Reductions use two custom DVE ops (registered below via the per-NEFF
ucode-table path; the native TENSOR_TENSOR_REDUCE opcode faults this
runtime's DVE sequencer): feeding the two 512-element halves of a block
as in0/in1 uses both SBUF read ports, so a 1024-element row reduces in
~512 DVE cycles (~0.69 us/block vs 1.21 for plain tensor_reduce). The
mean op applies scale=1/1024 inline, which lets the mean and max paths
share one unscaled FC1 weight block. All maxes plus 3/8 of the means
run on DVE (~123 us); the remaining means run on the scalar engine as
activation-Copy-accumulate (~121 us), so both reduce engines sit ~50 us
under the DMA stream and never backlog into the kernel tail.

The tiny grouped MLP uses block-diagonal duplicated weights
([[W.T, 0], [0, W.T]], built host-side): per group ONE 128x128x32 matmul
over the adjacent [mean|max] columns, relu, a DVE add of the two halves
(relu(mean@w1T) + relu(max@w1T)), one FC2 matmul with the summed hidden
as stationary, sigmoid, and a single batched store of all groups at the
end (the sync ring stays dedicated to the x stream).
"""

from operator import add as _op_add

import numpy as np

import concourse.bacc as bacc
import concourse.bass as bass
import concourse.dve_ops as dve_ops
import concourse.mybir as mybir
import concourse.tile as tile
from concourse.bass_utils import run_bass_kernel_spmd
from concourse.dve_spec import (
    C0,
    MaxNeg,
    Spec,
    Src0,
    Src1,
    Zero,
    _has_src1,
    lower,
    maxx,
)
from concourse.dve_uop import DveOpSpec

B, O, S, C, H, W = 8, 8, 32, 64, 32, 32
HID = C
HWSZ = H * W            # 1024 elements pooled per (b, o, s, c)
ROWS = O * S * C        # 16384 rows per core
RB = 128                # rows per partition block
T = ROWS // RB          # 128 row-blocks per core
JB = 8                  # row-blocks per stream tile (4 MiB DMAs)
HH = HWSZ // 2          # half-block length for the 2-port DVE reduce
SP = S // 2             # 16 pooled columns per (group, path)
N_CORES = 8

_CACHE = {}


# --- custom DVE pair-reduce ops -------------------------------------------


def _ref_pair_max(in0, in1, c0, c1, c2):
    b = np.maximum(in0.astype(np.float32), in1).astype(np.float32)
    return b, b.reshape(b.shape[0], -1).max(axis=-1, keepdims=True)


def _ref_pair_mean(in0, in1, c0, c1, c2):
    b = ((in0.astype(np.float32) + in1) * c0).astype(np.float32)
    return b, b.reshape(b.shape[0], -1).sum(axis=-1, keepdims=True)


def _make_op(name, spec):
    # Pin uops_sha by lowering locally for both ucode versions (the sha
    # covers only the serialized uop table bytes, not the opcode row).
    shas = {}
    for ver in ("v3", "v4"):
        tmp = DveOpSpec(
            name=name, opcode=0, uops=lower(spec, ver=ver), rd1_en=_has_src1(spec)
        )
        shas[ver] = tmp.sha(ver)
    return dve_ops.DveOp(name, spec, subdim=False, uops_sha=shas)


def _register_pair_ops():
    existing = {op.name: op for op in dve_ops.OPS}
    if "PAIR_MAX_REDUCE_ANT" in existing:
        return existing["PAIR_MAX_REDUCE_ANT"], existing["PAIR_MEAN_REDUCE_ANT"]
    pmax = _make_op(
        "PAIR_MAX_REDUCE_ANT",
        Spec(
            body=maxx(Src0, Src1),
            accum=maxx,
            accum_init=MaxNeg,
            reference=_ref_pair_max,
        ),
    )
    pmean = _make_op(
        "PAIR_MEAN_REDUCE_ANT",
        Spec(
            body=(Src0 + Src1) * C0,
            accum=_op_add,
            accum_init=Zero,
            reference=_ref_pair_mean,
        ),
    )
    for op in (pmax, pmean):
        dve_ops.OPS.append(op)
        dve_ops._SUB_OPCODE_FOR_NAME[op.name] = (
            max(dve_ops._SUB_OPCODE_FOR_NAME.values()) + 1
        )
        dve_ops.CUSTOM_DVE_SPECS[op.name] = op.spec
    assert max(dve_ops._SUB_OPCODE_FOR_NAME.values()) < 0x20
    return pmax, pmean


PMAX, PMEAN = _register_pair_ops()


def _build_nc():
    nc = bacc.Bacc(
        "TRN2", target_bir_lowering=False, debug=False, num_devices=N_CORES
    )
    x = nc.dram_tensor("x", [ROWS, HWSZ], mybir.dt.float32, kind="ExternalInput")
    wdup = nc.dram_tensor(
        "wdup", [128, 2 * O * 128], mybir.dt.float32, kind="ExternalInput"
    )
    out = nc.dram_tensor("out", [O * S, C], mybir.dt.float32, kind="ExternalOutput")

    fp32 = mybir.dt.float32
    AF = mybir.ActivationFunctionType
    ALU = mybir.AluOpType

    with tile.TileContext(nc) as tc:
        with (
            tc.tile_pool(name="xp", bufs=4) as xp,
            tc.tile_pool(name="tp", bufs=5) as tp,
            tc.tile_pool(name="small", bufs=1) as sp,
            tc.tile_pool(name="psum1", bufs=1, space=bass.MemorySpace.PSUM) as pp1,
            tc.tile_pool(name="psum2", bufs=1, space=bass.MemorySpace.PSUM) as pp2,
        ):
            # Weight load and the output store go through the scalar HWDGE
            # ring: the sync HWDGE ring executes FIFO per engine, so anything
            # else there would head-of-line block the x stream. GpSimd issues
            # no instructions at all.
            wd = sp.tile([128, 2 * O * 128], fp32)
            nc.scalar.dma_start(wd[:], wdup.ap())

            # pooled[:, o*32 + path*16 + j]: path 0 = mean, path 1 = max.
            pooled = sp.tile([128, 2 * T], fp32)
            junk = sp.tile([128, HWSZ], fp32)
            vjunk = sp.tile([128, HH], fp32)
            h_sb = sp.tile([128, 32 * O], fp32)
            hs_sb = sp.tile([128, SP * O], fp32)
            att = sp.tile([SP, O * 128], fp32)

            xv = x.ap().rearrange("(t p) f -> t p f", p=RB)
            # Output views for the stores: per partition j, one 512 B
            # descriptor per group (the r,c rows are contiguous in DRAM).
            ov = out.ap().rearrange("(o j r) c -> o j r c", o=O, j=SP, r=2)
            av = att.rearrange("p (o r c) -> o p r c", o=O, r=2)

            def colm(t):  # pooled column of block t's mean
                return (t // SP) * 32 + (t % SP)

            def colx(t):  # pooled column of block t's max
                return (t // SP) * 32 + SP + (t % SP)

            def dve_max(xt, j, t):
                nc.vector._custom_dve(
                    PMAX,
                    out=vjunk[:],
                    in0=xt[:, j, 0:HH],
                    in1=xt[:, j, HH:HWSZ],
                    accum_out=pooled[:, colx(t) : colx(t) + 1],
                )

            def dve_mean(xt, j, t):
                nc.vector._custom_dve(
                    PMEAN,
                    out=vjunk[:],
                    in0=xt[:, j, 0:HH],
                    in1=xt[:, j, HH:HWSZ],
                    s0=1.0 / HWSZ,
                    accum_out=pooled[:, colm(t) : colm(t) + 1],
                )

            def act_mean(xt, j, t):
                nc.scalar.activation(
                    junk[:],
                    xt[:, j, :],
                    AF.Copy,
                    scale=1.0 / HWSZ,
                    accum_out=pooled[:, colm(t) : colm(t) + 1],
                )

            def mlp(o):
                w1b = wd[:, o * 128 : (o + 1) * 128]
                w2b = wd[:, O * 128 + o * 128 : O * 128 + (o + 1) * 128]
                ps1 = pp1.tile([128, 32], fp32, tag="ps1")
                nc.tensor.matmul(ps1[:], w1b, pooled[:, o * 32 : (o + 1) * 32])
                h = h_sb[:, o * 32 : (o + 1) * 32]
                nc.scalar.activation(h, ps1[:], AF.Relu)
                hs = hs_sb[:, o * SP : (o + 1) * SP]
                nc.vector.tensor_tensor(hs, h[:, 0:SP], h[:, SP:32], op=ALU.add)
                ps2 = pp2.tile([SP, 128], fp32, tag="ps2")
                nc.tensor.matmul(ps2[:], hs, w2b)
                nc.scalar.activation(att[:, o * 128 : (o + 1) * 128], ps2[:], AF.Sigmoid)

            # 4 MiB DMAs for the bulk of the stream, tapering over the last
            # 16 blocks so the reduce engines never sit on more than one
            # small chunk of backlog when the final block lands (the
            # reduce->MLP->store chain starts almost immediately).
            chunks = [8] * 14 + [4, 4, 2, 2, 2, 1, 1]
            assert sum(chunks) == T
            t0 = 0
            for i, jb in enumerate(chunks):
                # Small taper chunks use their own (fresh) pool so their
                # DMAs are not gated on the big chunks' reduce bursts
                # freeing buffers; the 4-block taper chunks reuse xp slots
                # whose prior reads finished long before.
                if jb >= 4:
                    xt = xp.tile([RB, JB, HWSZ], fp32, tag="xt")
                else:
                    xt = tp.tile([RB, 2, HWSZ], fp32, tag="xtt")
                nc.sync.dma_start(
                    xt[:, :jb, :],
                    xv[t0 : t0 + jb].transpose([1, 0, 2]),
                )
                for j in range(jb):
                    dve_max(xt, j, t0 + j)
                # Means: DVE takes 3 of 8 per full chunk (2 of 8 late in the
                # stream, so its queue is drained when the taper arrives) and
                # one per 2-block taper chunk; the scalar engine takes the
                # rest, including the final single blocks so the last block's
                # mean and max run on different engines in parallel.
                for j in range(jb):
                    t = t0 + j
                    if (
                        (jb == 8 and j % 3 == 0)
                        or (jb == 4 and j % 2 == 0)
                        or (jb == 2 and j == 0)
                        or t == T - 2
                    ):
                        # Taper chunks put half the means on DVE (its per-op
                        # cost is 0.73 us vs 1.43 on ACT, and it also carries
                        # all the maxes); t == T-2 goes to DVE so the final
                        # block's mean (ACT) and max (DVE) finish in parallel
                        # right at stream end.
                        dve_mean(xt, j, t)
                    else:
                        act_mean(xt, j, t)
                # Group o's pooled columns are complete once row-blocks
                # through t = (o+1)*SP - 1 are reduced; emit its MLP as soon
                # as that happens so it overlaps the remaining stream.
                done = t0 + jb
                for o in range(O):
                    if t0 < (o + 1) * SP <= done:
                        mlp(o)
                        # Stores ride the scalar HWDGE ring (lower completion
                        # latency than SWDGE; the sync ring stays dedicated
                        # to the x stream). Groups 0-6 go out in one batched
                        # store that overlaps the tail of the stream; only
                        # group 7's 8 KiB store sits on the critical path
                        # after the final sigmoid.
                        if o == 6:
                            nc.scalar.dma_start(
                                ov[0:7].transpose([1, 0, 2, 3]),
                                av[0:7].transpose([1, 0, 2, 3]),
                            )
                        elif o == 7:
                            nc.scalar.dma_start(ov[7], av[7])
                t0 = done

    nc.compile()
    return nc


def _build_wdup(w1, w2):
    # Two sections of 8 block-diagonal duplicated 128x128 matrices: w1.T
    # (shared by the mean and max paths -- the mean is pre-scaled during
    # reduction) and w2.T.
    wdup = np.zeros((128, 2 * O * 128), dtype=np.float32)
    for o in range(O):
        w1t = np.ascontiguousarray(w1[o].T)  # [C, HID]
        w2t = np.ascontiguousarray(w2[o].T)  # [HID, C]
        for sec, blk in ((0, w1t), (1, w2t)):
            base = sec * O * 128 + o * 128
            wdup[0:64, base : base + 64] = blk
            wdup[64:128, base + 64 : base + 128] = blk
    return wdup


def kernel(x, w1, w2):
    if "nc" not in _CACHE:
        _CACHE["nc"] = _build_nc()
    nc = _CACHE["nc"]

    x = np.ascontiguousarray(x, dtype=np.float32).reshape(B, ROWS, HWSZ)
    wdup = _build_wdup(
        np.asarray(w1, dtype=np.float32), np.asarray(w2, dtype=np.float32)
    )
    in_maps = [{"x": x[b], "wdup": wdup} for b in range(B)]
    res = run_bass_kernel_spmd(nc, in_maps, core_ids=list(range(N_CORES)))
    out = np.stack([res.results[b]["out"] for b in range(B)])
    return out.reshape(B, O, S, C, 1, 1).astype(np.float32)


# revision 21
# speedup vs baseline: 1.0157x; 1.0031x over previous
"""ChannelAttention Trainium2 kernel.

Reference computation (per batch b, group o):
    p_mean[s, c] = mean over (h, w) of x[b, o, s, c, :, :]
    p_max[s, c]  = max  over (h, w) of x[b, o, s, c, :, :]
    out = sigmoid(relu(p_mean @ w1[o].T) @ w2[o].T + relu(p_max @ w1[o].T) @ w2[o].T)
    result[b, o, s, c, 0, 0] = out[s, c]

Strategy: data-parallel over batch B=8 -> one batch per NeuronCore (64 MiB
of x per core; the kernel is HBM-bandwidth bound on streaming x, ~392 GB/s
sustained per core = ~171 us).

Per core, x[b] is viewed as [O*S*C, H*W] = [16384, 1024] and streamed in
4 MiB tiles of [128 partitions, 8*1024] on the sync HWDGE ring. Each
128-row block covers 2 s-values x 64 channels, so pooled results land as
[partition = (s%2)*64 + c, column o*32 + path*16 + s//2] (path 0 = mean,
1 = max, adjacent so one matmul consumes both).

Reductions use two custom DVE ops (registered below via the per-NEFF
ucode-table path; the native TENSOR_TENSOR_REDUCE opcode faults this
runtime's DVE sequencer): feeding the two 512-element halves of a block
as in0/in1 uses both SBUF read ports, so a 1024-element row reduces in
~512 DVE cycles (~0.69 us/block vs 1.21 for plain tensor_reduce). The
mean op applies scale=1/1024 inline, which lets the mean and max paths
share one unscaled FC1 weight block. All maxes plus 3/8 of the means
run on DVE (~123 us); the remaining means run on the scalar engine as
activation-Copy-accumulate (~121 us), so both reduce engines sit ~50 us
under the DMA stream and never backlog into the kernel tail.

The tiny grouped MLP uses block-diagonal duplicated weights
([[W.T, 0], [0, W.T]], built host-side): per group ONE 128x128x32 matmul
over the adjacent [mean|max] columns, relu, a DVE add of the two halves
(relu(mean@w1T) + relu(max@w1T)), one FC2 matmul with the summed hidden
as stationary, sigmoid, and a single batched store of all groups at the
end (the sync ring stays dedicated to the x stream).
"""

from operator import add as _op_add

import numpy as np

import concourse.bacc as bacc
import concourse.bass as bass
import concourse.dve_ops as dve_ops
import concourse.mybir as mybir
import concourse.tile as tile
from concourse.bass_utils import run_bass_kernel_spmd
from concourse.dve_spec import (
    C0,
    MaxNeg,
    Spec,
    Src0,
    Src1,
    Zero,
    _has_src1,
    lower,
    maxx,
)
from concourse.dve_uop import DveOpSpec

B, O, S, C, H, W = 8, 8, 32, 64, 32, 32
HID = C
HWSZ = H * W            # 1024 elements pooled per (b, o, s, c)
ROWS = O * S * C        # 16384 rows per core
RB = 128                # rows per partition block
T = ROWS // RB          # 128 row-blocks per core
JB = 8                  # row-blocks per stream tile (4 MiB DMAs)
HH = HWSZ // 2          # half-block length for the 2-port DVE reduce
SP = S // 2             # 16 pooled columns per (group, path)
N_CORES = 8

_CACHE = {}


# --- custom DVE pair-reduce ops -------------------------------------------


def _ref_pair_max(in0, in1, c0, c1, c2):
    b = np.maximum(in0.astype(np.float32), in1).astype(np.float32)
    return b, b.reshape(b.shape[0], -1).max(axis=-1, keepdims=True)


def _ref_pair_mean(in0, in1, c0, c1, c2):
    b = ((in0.astype(np.float32) + in1) * c0).astype(np.float32)
    return b, b.reshape(b.shape[0], -1).sum(axis=-1, keepdims=True)


def _make_op(name, spec):
    # Pin uops_sha by lowering locally for both ucode versions (the sha
    # covers only the serialized uop table bytes, not the opcode row).
    shas = {}
    for ver in ("v3", "v4"):
        tmp = DveOpSpec(
            name=name, opcode=0, uops=lower(spec, ver=ver), rd1_en=_has_src1(spec)
        )
        shas[ver] = tmp.sha(ver)
    return dve_ops.DveOp(name, spec, subdim=False, uops_sha=shas)


def _register_pair_ops():
    existing = {op.name: op for op in dve_ops.OPS}
    if "PAIR_MAX_REDUCE_ANT" in existing:
        return existing["PAIR_MAX_REDUCE_ANT"], existing["PAIR_MEAN_REDUCE_ANT"]
    pmax = _make_op(
        "PAIR_MAX_REDUCE_ANT",
        Spec(
            body=maxx(Src0, Src1),
            accum=maxx,
            accum_init=MaxNeg,
            reference=_ref_pair_max,
        ),
    )
    pmean = _make_op(
        "PAIR_MEAN_REDUCE_ANT",
        Spec(
            body=(Src0 + Src1) * C0,
            accum=_op_add,
            accum_init=Zero,
            reference=_ref_pair_mean,
        ),
    )
    for op in (pmax, pmean):
        dve_ops.OPS.append(op)
        dve_ops._SUB_OPCODE_FOR_NAME[op.name] = (
            max(dve_ops._SUB_OPCODE_FOR_NAME.values()) + 1
        )
        dve_ops.CUSTOM_DVE_SPECS[op.name] = op.spec
    assert max(dve_ops._SUB_OPCODE_FOR_NAME.values()) < 0x20
    return pmax, pmean


PMAX, PMEAN = _register_pair_ops()


def _build_nc():
    nc = bacc.Bacc(
        "TRN2", target_bir_lowering=False, debug=False, num_devices=N_CORES
    )
    x = nc.dram_tensor("x", [ROWS, HWSZ], mybir.dt.float32, kind="ExternalInput")
    wdup = nc.dram_tensor(
        "wdup", [128, 2 * O * 128], mybir.dt.float32, kind="ExternalInput"
    )
    out = nc.dram_tensor("out", [O * S, C], mybir.dt.float32, kind="ExternalOutput")

    fp32 = mybir.dt.float32
    AF = mybir.ActivationFunctionType
    ALU = mybir.AluOpType

    with tile.TileContext(nc) as tc:
        with (
            tc.tile_pool(name="xp", bufs=4) as xp,
            tc.tile_pool(name="tp", bufs=5) as tp,
            tc.tile_pool(name="small", bufs=1) as sp,
            tc.tile_pool(name="psum1", bufs=1, space=bass.MemorySpace.PSUM) as pp1,
            tc.tile_pool(name="psum2", bufs=1, space=bass.MemorySpace.PSUM) as pp2,
        ):
            # Weight load and the output store go through the scalar HWDGE
            # ring: the sync HWDGE ring executes FIFO per engine, so anything
            # else there would head-of-line block the x stream. GpSimd issues
            # no instructions at all.
            wd = sp.tile([128, 2 * O * 128], fp32)
            nc.scalar.dma_start(wd[:], wdup.ap())

            # pooled[:, o*32 + path*16 + j]: path 0 = mean, path 1 = max.
            pooled = sp.tile([128, 2 * T], fp32)
            junk = sp.tile([128, HWSZ], fp32)
            vjunk = sp.tile([128, HH], fp32)
            h_sb = sp.tile([128, 32 * O], fp32)
            hs_sb = sp.tile([128, SP * O], fp32)
            att = sp.tile([SP, O * 128], fp32)

            xv = x.ap().rearrange("(t p) f -> t p f", p=RB)
            # Output views for the stores: per partition j, one 512 B
            # descriptor per group (the r,c rows are contiguous in DRAM).
            ov = out.ap().rearrange("(o j r) c -> o j r c", o=O, j=SP, r=2)
            av = att.rearrange("p (o r c) -> o p r c", o=O, r=2)

            def colm(t):  # pooled column of block t's mean
                return (t // SP) * 32 + (t % SP)

            def colx(t):  # pooled column of block t's max
                return (t // SP) * 32 + SP + (t % SP)

            def dve_max(xt, j, t):
                nc.vector._custom_dve(
                    PMAX,
                    out=vjunk[:],
                    in0=xt[:, j, 0:HH],
                    in1=xt[:, j, HH:HWSZ],
                    accum_out=pooled[:, colx(t) : colx(t) + 1],
                )

            def dve_mean(xt, j, t):
                nc.vector._custom_dve(
                    PMEAN,
                    out=vjunk[:],
                    in0=xt[:, j, 0:HH],
                    in1=xt[:, j, HH:HWSZ],
                    s0=1.0 / HWSZ,
                    accum_out=pooled[:, colm(t) : colm(t) + 1],
                )

            def act_mean(xt, j, t):
                nc.scalar.activation(
                    junk[:],
                    xt[:, j, :],
                    AF.Copy,
                    scale=1.0 / HWSZ,
                    accum_out=pooled[:, colm(t) : colm(t) + 1],
                )

            def mlp(o):
                w1b = wd[:, o * 128 : (o + 1) * 128]
                w2b = wd[:, O * 128 + o * 128 : O * 128 + (o + 1) * 128]
                ps1 = pp1.tile([128, 32], fp32, tag="ps1")
                nc.tensor.matmul(ps1[:], w1b, pooled[:, o * 32 : (o + 1) * 32])
                h = h_sb[:, o * 32 : (o + 1) * 32]
                nc.scalar.activation(h, ps1[:], AF.Relu)
                hs = hs_sb[:, o * SP : (o + 1) * SP]
                nc.vector.tensor_tensor(hs, h[:, 0:SP], h[:, SP:32], op=ALU.add)
                ps2 = pp2.tile([SP, 128], fp32, tag="ps2")
                nc.tensor.matmul(ps2[:], hs, w2b)
                nc.scalar.activation(att[:, o * 128 : (o + 1) * 128], ps2[:], AF.Sigmoid)

            # 4 MiB DMAs for the bulk of the stream, tapering over the last
            # 16 blocks so the reduce engines never sit on more than one
            # small chunk of backlog when the final block lands (the
            # reduce->MLP->store chain starts almost immediately).
            chunks = [8] * 14 + [4, 4, 2, 2, 2, 1, 1]
            assert sum(chunks) == T
            t0 = 0
            for i, jb in enumerate(chunks):
                # Small taper chunks use their own (fresh) pool so their
                # DMAs are not gated on the big chunks' reduce bursts
                # freeing buffers; the 4-block taper chunks reuse xp slots
                # whose prior reads finished long before.
                if jb >= 4:
                    xt = xp.tile([RB, JB, HWSZ], fp32, tag="xt")
                else:
                    xt = tp.tile([RB, 2, HWSZ], fp32, tag="xtt")
                nc.sync.dma_start(
                    xt[:, :jb, :],
                    xv[t0 : t0 + jb].transpose([1, 0, 2]),
                )
                for j in range(jb):
                    dve_max(xt, j, t0 + j)
                # Means: DVE takes 3 of 8 per full chunk (2 of 8 late in the
                # stream, so its queue is drained when the taper arrives) and
                # one per 2-block taper chunk; the scalar engine takes the
                # rest, including the final single blocks so the last block's
                # mean and max run on different engines in parallel.
                for j in range(jb):
                    t = t0 + j
                    if (
                        (jb == 8 and j % 3 == 0)
                        or (jb == 4 and j == 0)
                        or t == T - 2
                    ):
                        # In the taper DVE already carries all 16 maxes
                        # (11.7 us of its ~21 us window), so most taper means
                        # go to ACT, which has slack; t == T-2 goes to DVE so
                        # the final block's mean (ACT) and max (DVE) finish
                        # in parallel right at stream end.
                        dve_mean(xt, j, t)
                    else:
                        act_mean(xt, j, t)
                # Group o's pooled columns are complete once row-blocks
                # through t = (o+1)*SP - 1 are reduced; emit its MLP as soon
                # as that happens so it overlaps the remaining stream.
                done = t0 + jb
                for o in range(O):
                    if t0 < (o + 1) * SP <= done:
                        mlp(o)
                        # Stores ride the scalar HWDGE ring (lower completion
                        # latency than SWDGE; the sync ring stays dedicated
                        # to the x stream). Groups 0-6 go out in one batched
                        # store that overlaps the tail of the stream; only
                        # group 7's 8 KiB store sits on the critical path
                        # after the final sigmoid.
                        if o == 6:
                            nc.scalar.dma_start(
                                ov[0:7].transpose([1, 0, 2, 3]),
                                av[0:7].transpose([1, 0, 2, 3]),
                            )
                        elif o == 7:
                            nc.scalar.dma_start(ov[7], av[7])
                t0 = done

    nc.compile()
    return nc


def _build_wdup(w1, w2):
    # Two sections of 8 block-diagonal duplicated 128x128 matrices: w1.T
    # (shared by the mean and max paths -- the mean is pre-scaled during
    # reduction) and w2.T.
    wdup = np.zeros((128, 2 * O * 128), dtype=np.float32)
    for o in range(O):
        w1t = np.ascontiguousarray(w1[o].T)  # [C, HID]
        w2t = np.ascontiguousarray(w2[o].T)  # [HID, C]
        for sec, blk in ((0, w1t), (1, w2t)):
            base = sec * O * 128 + o * 128
            wdup[0:64, base : base + 64] = blk
            wdup[64:128, base + 64 : base + 128] = blk
    return wdup


def kernel(x, w1, w2):
    if "nc" not in _CACHE:
        _CACHE["nc"] = _build_nc()
    nc = _CACHE["nc"]

    x = np.ascontiguousarray(x, dtype=np.float32).reshape(B, ROWS, HWSZ)
    wdup = _build_wdup(
        np.asarray(w1, dtype=np.float32), np.asarray(w2, dtype=np.float32)
    )
    in_maps = [{"x": x[b], "wdup": wdup} for b in range(B)]
    res = run_bass_kernel_spmd(nc, in_maps, core_ids=list(range(N_CORES)))
    out = np.stack([res.results[b]["out"] for b in range(B)])
    return out.reshape(B, O, S, C, 1, 1).astype(np.float32)
